# revision 1
# baseline (speedup 1.0000x reference)
"""im2col (3x3, SAME zero padding) kernel for Trainium2.

Full op: x (16, 64, 128, 128) f32 -> out (16, 128, 128, 64, 3, 3) f32 with
    out[b, h, w, c, i, j] = pad(x)[b, c, h + i, w + j]   (pad = 1 px zeros)

Sharding: data-parallel over batch. 8 cores x 2 batches each, no
cross-device communication.

Per-core kernel structure (Tile framework):
  1. Stream x[b] in 32-row chunks into SBUF laid out (64c, (CH+2) rows x 130)
     with one zero column on each side of every row (and zero halo rows at
     the image top/bottom), so all nine shifted reads become plain AP
     offsets with the boundary zeros already materialized.
  2. For each padded row, 3 TensorE transposes (lhsT = the (64, 128) row
     window at w-offset j, rhs = 64x64 identity) -> PSUM (128w, 64c),
     packed as (128, 192) = [j][c].
  3. One PSUM->SBUF copy per row stages xT[(row)(j)(c)].
  4. Per output row h: 3 interleave copies (one per j, i-fused via 2D APs)
     write the final (w, [c,3,3]) layout; copies are split across
     DVE / GPSIMD / ACT so no single engine is the bottleneck.
  5. One ~1.2 MB DMA stores G=4 output rows (contiguous 2304 B per (h,w)).
"""

import sys

for _p in ("/opt/trn_rl_repo", "/root/.axon_site/_ro/trn_rl_repo"):
    if _p not in sys.path:
        sys.path.append(_p)

import numpy as np

import concourse.bacc as bacc
import concourse.mybir as mybir
from concourse import bass_utils, masks
from concourse.tile import TileContext

F32 = mybir.dt.float32

# Problem shape (hardcoded; the grading harness provides exactly this).
B, C, H, W = 16, 64, 128, 128
KS = 3  # kernel size
N_CORES = 8
B_LOC = B // N_CORES  # batches per core

WP = W + 2  # padded row length
CH = 32  # h-chunk size
CHP = CH + 2  # padded rows per chunk
G = 4  # output rows per store DMA


def _build_kernel(n_b: int = B_LOC):
    nc = bacc.Bacc("TRN2", target_bir_lowering=False, debug=False)

    x = nc.dram_tensor("x", (n_b, C, H, W), F32, kind="ExternalInput")
    out = nc.dram_tensor("out", (n_b, H, W, C, KS, KS), F32, kind="ExternalOutput")
    x_ap = x.ap()
    out_ap = out.ap()

    with TileContext(nc) as tc:
        with (
            tc.tile_pool(name="const", bufs=1) as const_pool,
            tc.tile_pool(name="xin", bufs=2) as xin_pool,
            tc.tile_pool(name="xt", bufs=2) as xt_pool,
            tc.tile_pool(name="ps", bufs=4, space="PSUM") as psum_pool,
            tc.tile_pool(name="osb", bufs=4) as out_pool,
        ):
            ident = const_pool.tile([C, C], F32)
            masks.make_identity(nc, ident)

            copy_engines = [nc.vector.tensor_copy, nc.scalar.copy]

            for b in range(n_b):
                for h0 in range(0, H, CH):
                    # ---- load chunk: padded rows h0 .. h0+CHP-1 (global
                    # unpadded rows h0-1 .. h0+CH) ----
                    xin = xin_pool.tile([C, CHP * WP], F32)
                    xin_r = xin.rearrange("p (r q) -> p r q", q=WP)
                    # zero pad columns (w = -1 and w = W)
                    nc.vector.memset(xin_r[:, :, 0:1], 0.0)
                    nc.vector.memset(xin_r[:, :, WP - 1 : WP], 0.0)
                    g_lo = h0 - 1
                    lo = 0
                    n_rows = CHP
                    if g_lo < 0:  # top halo row is out of image -> zeros
                        nc.vector.memset(xin_r[:, 0:1, :], 0.0)
                        g_lo, lo, n_rows = 0, 1, n_rows - 1
                    if h0 + CH + 1 > H:  # bottom halo row -> zeros
                        nc.vector.memset(xin_r[:, CHP - 1 : CHP, :], 0.0)
                        n_rows -= 1
                    nc.sync.dma_start(
                        out=xin_r[:, lo : lo + n_rows, 1 : W + 1],
                        in_=x_ap[b, :, g_lo : g_lo + n_rows, :],
                    )

                    # ---- transpose every padded row, 3 w-shifts each ----
                    xt = xt_pool.tile([W, CHP * KS * C], F32)
                    for li in range(CHP):
                        ps = psum_pool.tile([W, KS * C], F32)
                        for j in range(KS):
                            nc.tensor.transpose(
                                ps[:, j * C : (j + 1) * C],
                                xin_r[:, li, j : j + W],
                                ident,
                            )
                        # stage PSUM -> SBUF (alternate DVE / ACT)
                        copy_engines[li % 2](
                            xt[:, li * KS * C : (li + 1) * KS * C], ps
                        )

                    # ---- assemble + store, G output rows per DMA ----
                    xt_r = xt.rearrange("p (r j c) -> p r j c", j=KS, c=C)
                    for hg in range(0, CH, G):
                        osb = out_pool.tile([W, G * C * KS * KS], F32)
                        # dims: (p, g, i, c, j) so copies see (p, i, c)
                        osb_v = osb.rearrange(
                            "p (g c i j) -> p g i c j", g=G, c=C, i=KS, j=KS
                        )
                        for hs in range(G):
                            hl = hg + hs  # chunk-local output row
                            for j in range(KS):
                                src = xt_r[:, hl : hl + KS, j, :]  # (p, i, c)
                                dst = osb_v[:, hs, :, :, j]  # (p, i, c)
                                if j == 0:
                                    nc.vector.tensor_copy(dst, src)
                                elif j == 1:
                                    nc.gpsimd.tensor_copy(dst, src)
                                else:
                                    nc.scalar.copy(dst, src)
                        nc.sync.dma_start(
                            out=out_ap[b].rearrange("h w c i j -> w h (c i j)")[
                                :, h0 + hg : h0 + hg + G, :
                            ],
                            in_=osb.rearrange("p (g f) -> p g f", f=C * KS * KS),
                        )

    nc.compile()
    return nc


_NC_CACHE = {}


def _get_nc(n_b: int):
    if n_b not in _NC_CACHE:
        _NC_CACHE[n_b] = _build_kernel(n_b)
    return _NC_CACHE[n_b]


def run_spmd(x: np.ndarray, **kwargs) -> bass_utils.BassKernelResults:
    """Run the SPMD kernel on 8 cores; returns raw BassKernelResults."""
    x = np.ascontiguousarray(np.asarray(x, dtype=np.float32))
    assert x.shape == (B, C, H, W), x.shape
    nc = _get_nc(B_LOC)
    in_maps = [
        {"x": x[i * B_LOC : (i + 1) * B_LOC]} for i in range(N_CORES)
    ]
    return bass_utils.run_bass_kernel_spmd(
        nc, in_maps, core_ids=list(range(N_CORES)), **kwargs
    )


def kernel(x: np.ndarray) -> np.ndarray:
    res = run_spmd(x)
    return np.concatenate([r["out"] for r in res.results], axis=0)


# revision 4
# speedup vs baseline: 534.3914x; 534.3914x over previous
"""im2col (3x3, SAME zero padding) kernel for Trainium2.

Full op: x (16, 64, 128, 128) f32 -> out (16, 128, 128, 64, 3, 3) f32 with
    out[b, h, w, c, i, j] = pad(x)[b, c, h + i, w + j]   (pad = 1 px zeros)

Sharding: data-parallel over batch. 8 cores x 2 batches each, no
cross-device communication.

Per-core kernel structure (Tile framework):
  1. Stream x[b] in 32-row chunks into SBUF laid out (64c, (CH+2) rows x 130)
     with one zero column on each side of every row (and zero halo rows at
     the image top/bottom), so all nine shifted reads become plain AP
     offsets with the boundary zeros already materialized.
  2. For each padded row, 3 TensorE transposes (lhsT = the (64, 128) row
     window at w-offset j, rhs = 64x64 identity) -> PSUM (128w, 64c),
     packed as (128, 192) = [j][c].
  3. One PSUM->SBUF copy per row stages xT[(row)(j)(c)].
  4. Per output row h: 3 interleave copies (one per j, i-fused via 2D APs)
     write the final (w, [c,3,3]) layout; copies are split across
     DVE / GPSIMD / ACT so no single engine is the bottleneck.
  5. One ~1.2 MB DMA stores G=4 output rows (contiguous 2304 B per (h,w)).
"""

import sys

for _p in ("/opt/trn_rl_repo", "/root/.axon_site/_ro/trn_rl_repo"):
    if _p not in sys.path:
        sys.path.append(_p)

import numpy as np

import concourse.bacc as bacc
import concourse.mybir as mybir
from concourse import bass_utils, masks
from concourse.tile import TileContext

F32 = mybir.dt.float32

# Problem shape (hardcoded; the grading harness provides exactly this).
B, C, H, W = 16, 64, 128, 128
KS = 3  # kernel size
N_CORES = 8
B_LOC = B // N_CORES  # batches per core

WP = W + 2  # padded row length
CH = 32  # h-chunk size
CHP = CH + 2  # padded rows per chunk
G = 4  # output rows per store DMA


def _build_kernel(n_b: int = B_LOC, repeat: int = 1, g: int = G):
    nc = bacc.Bacc("TRN2", target_bir_lowering=False, debug=False)

    x = nc.dram_tensor("x", (n_b, C, H, W), F32, kind="ExternalInput")
    out = nc.dram_tensor("out", (n_b, H, W, C, KS, KS), F32, kind="ExternalOutput")
    x_ap = x.ap()
    out_ap = out.ap()

    with TileContext(nc) as tc:
        with (
            tc.tile_pool(name="const", bufs=1) as const_pool,
            tc.tile_pool(name="xin", bufs=2) as xin_pool,
            tc.tile_pool(name="xt", bufs=2) as xt_pool,
            tc.tile_pool(name="ps", bufs=4, space="PSUM") as psum_pool,
            tc.tile_pool(name="osb", bufs=(4 if g <= 4 else 3)) as out_pool,
        ):
            ident = const_pool.tile([C, C], F32)
            masks.make_identity(nc, ident)

            copy_engines = [nc.vector.tensor_copy, nc.scalar.copy]

            for _rep in range(repeat):
              for b in range(n_b):
                for h0 in range(0, H, CH):
                    # ---- load chunk: padded rows h0 .. h0+CHP-1 (global
                    # unpadded rows h0-1 .. h0+CH) ----
                    xin = xin_pool.tile([C, CHP * WP], F32)
                    xin_r = xin.rearrange("p (r q) -> p r q", q=WP)
                    # zero pad columns (w = -1 and w = W)
                    nc.vector.memset(xin_r[:, :, 0:1], 0.0)
                    nc.vector.memset(xin_r[:, :, WP - 1 : WP], 0.0)
                    g_lo = h0 - 1
                    lo = 0
                    n_rows = CHP
                    if g_lo < 0:  # top halo row is out of image -> zeros
                        nc.vector.memset(xin_r[:, 0:1, :], 0.0)
                        g_lo, lo, n_rows = 0, 1, n_rows - 1
                    if h0 + CH + 1 > H:  # bottom halo row -> zeros
                        nc.vector.memset(xin_r[:, CHP - 1 : CHP, :], 0.0)
                        n_rows -= 1
                    nc.sync.dma_start(
                        out=xin_r[:, lo : lo + n_rows, 1 : W + 1],
                        in_=x_ap[b, :, g_lo : g_lo + n_rows, :],
                    )

                    # ---- transpose every padded row, 3 w-shifts each ----
                    xt = xt_pool.tile([W, CHP * KS * C], F32)
                    for li in range(CHP):
                        ps = psum_pool.tile([W, KS * C], F32)
                        for j in range(KS):
                            nc.tensor.transpose(
                                ps[:, j * C : (j + 1) * C],
                                xin_r[:, li, j : j + W],
                                ident,
                            )
                        # stage PSUM -> SBUF (alternate DVE / ACT)
                        copy_engines[li % 2](
                            xt[:, li * KS * C : (li + 1) * KS * C], ps
                        )

                    # ---- assemble + store, G output rows per DMA ----
                    xt_r = xt.rearrange("p (r j c) -> p r j c", j=KS, c=C)
                    for hg in range(0, CH, g):
                        osb = out_pool.tile([W, g * C * KS * KS], F32)
                        # dims: (p, g, i, c, j) so copies see (p, i, c)
                        osb_v = osb.rearrange(
                            "p (g c i j) -> p g i c j", g=g, c=C, i=KS, j=KS
                        )
                        for hs in range(g):
                            hl = hg + hs  # chunk-local output row
                            for j in range(KS):
                                src = xt_r[:, hl : hl + KS, j, :]  # (p, i, c)
                                dst = osb_v[:, hs, :, :, j]  # (p, i, c)
                                if j == 0:
                                    nc.vector.tensor_copy(dst, src)
                                elif j == 1:
                                    nc.gpsimd.tensor_copy(dst, src)
                                else:
                                    nc.scalar.copy(dst, src)
                        nc.sync.dma_start(
                            out=out_ap[b].rearrange("h w c i j -> w h (c i j)")[
                                :, h0 + hg : h0 + hg + g, :
                            ],
                            in_=osb.rearrange("p (g f) -> p g f", f=C * KS * KS),
                        )

    nc.compile()
    return nc


_NC_CACHE = {}


def _get_nc(n_b: int):
    if n_b not in _NC_CACHE:
        _NC_CACHE[n_b] = _build_kernel(n_b)
    return _NC_CACHE[n_b]


def run_spmd(x: np.ndarray, **kwargs) -> bass_utils.BassKernelResults:
    """Run the SPMD kernel on 8 cores; returns raw BassKernelResults."""
    x = np.ascontiguousarray(np.asarray(x, dtype=np.float32))
    assert x.shape == (B, C, H, W), x.shape
    nc = _get_nc(B_LOC)
    in_maps = [
        {"x": x[i * B_LOC : (i + 1) * B_LOC]} for i in range(N_CORES)
    ]
    return bass_utils.run_bass_kernel_spmd(
        nc, in_maps, core_ids=list(range(N_CORES)), **kwargs
    )


def kernel(x: np.ndarray) -> np.ndarray:
    res = run_spmd(x)
    return np.concatenate([r["out"] for r in res.results], axis=0)
